# revision 31
# baseline (speedup 1.0000x reference)
"""Trainium2 Bass kernel for nn_CausalSelfAttention_22127671509246.

Full (unsharded) inputs in, full output out. Internally shards across 8
NeuronCores: core c handles batch b = c // 4 and head group g = c % 4
(heads 4g..4g+3, i.e. a 256-wide slice of the QKV output channels).

v3 design (per core, 4 heads = 2 head pairs):
  - Q^T/K^T projections into flat [128,512] chunks (channel-major), V
    projection row-major in fp16 with a ones column per head (PV matmul
    then also produces the softmax denominator).
  - attention blocks (pair, qc): QK in 64-row matmuls (2 cols/cycle via
    row replication); logits exp'd on TWO engines:
      * 9/16 k-tiles: ScalarE Exp activation -> bf16
      * 7/16 k-tiles: DVE Schraudolph (i16 = round(l*A+B), bitcast fp16
        ~= exp(l); softmax normalization cancels most of the ~3% elem
        error -> ~0.9% output err)
  - PV matmuls of block i-1 interleave per-kp-step into block i's QK
    stream; per-head epilogues at steps 3/7 release PSUM early.
  - ALL projection work (V halves, ct1 Q/K, late Q ct0) is spread as
    per-step PE filler so the PE never idles and the HAM clock-gate
    stays at 2.4 GHz; junk matmuls keep it warm when filler runs dry.
"""

import sys
import types
from collections import deque

sys.path.insert(0, "/opt/trn_rl_repo")

import numpy as np
import ml_dtypes

import concourse.bass as bass
import concourse.bacc as bacc
import concourse.mybir as mybir
import concourse.tile as tile
from concourse.bass import ts

B, S, D = 2, 2048, 1024
H, HD = 16, 64
N_CORES = 8
C = 256           # output channels per core (4 heads)
CT = C // 128     # channel tiles per core
KD = D // 128     # contraction chunks for the projections
SC = S // 512     # 512-wide column chunks of S
STL = S // 128    # 128-row tiles of S
HPC = 4           # heads per core
SCALE = 1.0 / np.sqrt(HD)

LOG2E = float(np.log2(np.e))
SCH_A = SCALE * LOG2E * 1024.0          # fold softmax scale into schraudolph
SCH_B = (15.0 - 0.043) * 1024.0         # fp16 bias + optimal shift

F32 = mybir.dt.float32
BF16 = mybir.dt.bfloat16
FP16 = mybir.dt.float16
I16 = mybir.dt.int16

# k-tiles handled by the DVE (schraudolph); the rest by ScalarE exp
DVE_KT = frozenset({1, 3, 5, 7, 11, 13})

_compiled = {}


def _install_ntff_hook():
    if "antenv.axon_hooks" in sys.modules:
        return
    try:
        import trn_agent_boot.trn_boot as tb

        mod = types.ModuleType("antenv.axon_hooks")
        hook = tb._ntff_profile_via_ctypes("/opt/axon/libaxon_pjrt.so")
        mod.get_axon_ntff_profile_hook = lambda: hook
        mod.set_axon_ntff_profile_hook = lambda h: None
        sys.modules["antenv.axon_hooks"] = mod
    except Exception:
        pass


def _emit(tc, ctx):
    nc = tc.nc
    xT = nc.dram_tensor("xT", [D, S], BF16, kind="ExternalInput").ap()
    wq = nc.dram_tensor("wq", [D, C], BF16, kind="ExternalInput").ap()
    wk = nc.dram_tensor("wk", [D, C], BF16, kind="ExternalInput").ap()
    wv = nc.dram_tensor("wv", [D, C], BF16, kind="ExternalInput").ap()
    bq = nc.dram_tensor("bq", [C], F32, kind="ExternalInput").ap()
    bk = nc.dram_tensor("bk", [C], F32, kind="ExternalInput").ap()
    bv = nc.dram_tensor("bv", [C], F32, kind="ExternalInput").ap()
    y = nc.dram_tensor("y", [S, C], F32, kind="ExternalOutput").ap()

    singles = ctx.enter_context(tc.tile_pool(name="singles", bufs=1))
    ax_pool = ctx.enter_context(tc.tile_pool(name="ax", bufs=34))
    yout_pool = ctx.enter_context(tc.tile_pool(name="yout", bufs=4))
    recip_pool = ctx.enter_context(tc.tile_pool(name="recip", bufs=4))
    ps_pool = ctx.enter_context(tc.tile_pool(name="ps", bufs=3, space="PSUM"))
    psy_pool = ctx.enter_context(tc.tile_pool(name="psy", bufs=1, space="PSUM"))

    # ---- input DMAs: fine chunks, alternating the two HWDGE queues ----
    xT_r = xT.rearrange("(o p) s -> p o s", p=128)
    xT_sb = singles.tile([128, KD, S], BF16)
    w_sbs = {
        "q": singles.tile([128, KD, C], BF16, tag="wq", name="wq_sb"),
        "k": singles.tile([128, KD, C], BF16, tag="wk", name="wk_sb"),
        "v": singles.tile([128, KD, C], BF16, tag="wv", name="wv_sb"),
    }
    bq_sb = singles.tile([128, CT], F32, tag="bq")
    bk_sb = singles.tile([128, CT], F32, tag="bk")

    # FEW input DMA transfers: each dma_start costs ~2us of issue +
    # completion-semaphore latency per queue, so the critical path must
    # cross as few semaphores as possible
    nc.sync.dma_start(w_sbs["k"][:], wk.rearrange("(o p) c -> p o c", p=128))
    nc.scalar.dma_start(w_sbs["q"][:], wq.rearrange("(o p) c -> p o c", p=128))
    nc.sync.dma_start(xT_sb[:, 0:2, :], xT_r[:, 0:2, :])
    nc.scalar.dma_start(xT_sb[:, 2:4, :], xT_r[:, 2:4, :])
    nc.sync.dma_start(xT_sb[:, 4:6, :], xT_r[:, 4:6, :])
    nc.scalar.dma_start(xT_sb[:, 6:8, :], xT_r[:, 6:8, :])
    nc.sync.dma_start(w_sbs["v"][:], wv.rearrange("(o p) c -> p o c", p=128))
    # strided many-descriptor bias loads go through the gpsimd software
    # DGE so they don't block the HWDGE queues' in-order completions
    nc.gpsimd.dma_start(out=bk_sb[:], in_=bk.rearrange("(o p) -> p o", p=128))
    nc.gpsimd.dma_start(out=bq_sb[:], in_=bq.rearrange("(o p) -> p o", p=128))
    # bv broadcast across partitions (DMA with partition step 0)
    bv_bc = singles.tile([128, C], F32, tag="bvbc")
    bv_bcast_ap = bass.AP(tensor=bv.tensor, offset=bv.offset,
                          ap=[[0, 128]] + list(bv.ap))
    nc.gpsimd.dma_start(out=bv_bc[:], in_=bv_bcast_ap)

    # ---- PE warm-up junk matmuls (HAM clock-gate to 8/8) ----
    junk = singles.tile([128, 512], BF16, tag="junk")
    nc.vector.memset(junk[:], 0.0)

    def junk_mms(n):
        ps = ps_pool.tile([128, 512], F32, tag="qk", name="warm")
        for r in range(n):
            nc.tensor.matmul(ps[:], lhsT=junk[:, 0:128], rhs=junk[:],
                             start=True, stop=True, skip_group_check=True)

    junk_mms(24)

    # V with a ones column appended per head: [128, s_tile, head, 65] fp16
    v_sb = singles.tile([128, STL, HPC, HD + 1], FP16, tag="vones")
    nc.vector.memset(v_sb[:, :, :, HD], 1.0)

    # flat projection chunk tiles
    qch = [[singles.tile([128, 512], BF16, tag=f"qc{ct}{sc}",
                         name=f"qc{ct}{sc}")
            for sc in range(SC)] for ct in range(CT)]
    kch = [[singles.tile([128, 512], BF16, tag=f"kc{ct}{sc}",
                         name=f"kc{ct}{sc}")
            for sc in range(SC)] for ct in range(CT)]

    def proj_qk_chain(which, ct, sc, eng="v"):
        w_sb = w_sbs[which]
        dst = (qch if which == "q" else kch)[ct][sc]
        bias = bq_sb if which == "q" else bk_sb
        ps = ps_pool.tile([128, 512], F32, tag="qk", name="ps_proj")
        for kd in range(KD):
            nc.tensor.matmul(
                ps[:],
                lhsT=w_sb[:, kd, ts(ct, 128)],
                rhs=xT_sb[:, kd, ts(sc, 512)],
                start=(kd == 0),
                stop=(kd == KD - 1),
            )
        if eng == "v":
            nc.vector.tensor_scalar_add(dst[:], ps[:], bias[:, ct:ct + 1])
        else:
            nc.scalar.activation(dst[:], ps[:],
                                 mybir.ActivationFunctionType.Identity,
                                 bias=bias[:, ct:ct + 1], scale=1.0)

    def proj_v_chain(st, pair):
        """V projection for one 128-row s-tile, one head PAIR (N=128)."""
        ps = ps_pool.tile([128, 128], F32, tag="qk", name="ps_projv")
        for kd in range(KD):
            nc.tensor.matmul(
                ps[:],
                lhsT=xT_sb[:, kd, ts(st, 128)],
                rhs=w_sbs["v"][:, kd, ts(pair, 128)],
                start=(kd == 0),
                stop=(kd == KD - 1),
            )
        nc.vector.tensor_tensor(
            v_sb[:, st, 2 * pair:2 * pair + 2, 0:HD],
            ps[:].rearrange("p (h d) -> p h d", h=2),
            bv_bc[:, ts(pair, 128)].rearrange("p (h d) -> p h d", h=2),
            mybir.AluOpType.add,
        )

    # ---- attention blocks ----
    blocks = [(pair, qc) for pair in range(HPC // 2) for qc in range(SC)]

    def qk_exp_block(pair, qc, ax_tiles):
        """Per kp step: QK(u0), exp(u0), QK(u1), exp(u1)."""
        ct = pair
        for kp in range(STL // 2):
            for u in range(2):
                kt = 2 * kp + u
                psu = ps_pool.tile([128, 1024], F32, tag="qk", name="ps_att")
                for hh in range(2):
                    p0 = hh * 64
                    nc.tensor.matmul(
                        psu[:, ts(hh, 512)],
                        lhsT=kch[ct][kt // 4][p0:p0 + 64, ts(kt % 4, 128)],
                        rhs=qch[ct][qc][p0:p0 + 64, :],
                        start=True,
                        stop=True,
                    )
                if kt in DVE_KT:
                    nc.vector.tensor_scalar(
                        ax_tiles[kt].bitcast(I16)[:], psu[:],
                        SCH_A, SCH_B,
                        mybir.AluOpType.mult, mybir.AluOpType.add,
                    )
                else:
                    nc.scalar.activation(
                        ax_tiles[kt][:], psu[:],
                        mybir.ActivationFunctionType.Exp, scale=SCALE,
                    )
            yield

    def pv_mms(pair, qc, ax_tiles, y_ps):
        mms = []
        for hh in range(2):
            h = 2 * pair + hh
            for j in range(4):
                for kt in range(STL):
                    def mm(hh=hh, h=h, j=j, kt=kt):
                        axt = ax_tiles[kt]
                        sl = slice(hh * 512 + j * 128,
                                   hh * 512 + (j + 1) * 128)
                        lhsT = (axt.bitcast(FP16)[:, sl] if kt in DVE_KT
                                else axt[:, sl])
                        nc.tensor.matmul(
                            y_ps[hh][:, j, :],
                            lhsT=lhsT,
                            rhs=v_sb[:, kt, h, :],
                            start=(kt == 0),
                            stop=(kt == STL - 1),
                        )
                    mms.append(mm)
        return mms

    def epilogue_h(pair, qc, y_ps, hh):
        h = 2 * pair + hh
        yo = yout_pool.tile([128, 4, HD], F32, tag="yo", name="yo")
        rc = recip_pool.tile([128, 4], F32, tag="rc", name="rc")
        nc.vector.reciprocal(rc[:], y_ps[hh][:, :, HD])
        nc.vector.tensor_tensor(
            yo[:],
            y_ps[hh][:, :, 0:HD],
            rc[:, :, None].to_broadcast((128, 4, HD)),
            mybir.AluOpType.mult,
        )
        nc.sync.dma_start(
            y[ts(qc, 512), ts(h, HD)].rearrange("(j p) d -> p j d", p=128),
            yo[:],
        )

    # lead-in: just K ct0 sc0 + Q ct0 sc0 (all block 0 needs to start)
    proj_qk_chain("k", 0, 0)
    proj_qk_chain("q", 0, 0)

    # per-block PE filler chains (each ~0.4-1.7us of matmul work).
    # In blocks with PV batches they run right after a 128-row PV batch so
    # they don't add extra 64<->128-row mode transitions.
    filler = {i: deque() for i in range(len(blocks))}
    filler[0].append(lambda: proj_qk_chain("k", 0, 1, "s"))
    filler[0].append(lambda: proj_qk_chain("q", 0, 1, "v"))
    filler[0].append(lambda: proj_qk_chain("k", 0, 2, "s"))
    filler[0].append(lambda: proj_qk_chain("k", 0, 3, "v"))
    for st in range(6):
        filler[0].append(lambda st=st: proj_v_chain(st, 0))
    filler[1].append(lambda: proj_qk_chain("q", 0, 2, "s"))
    filler[1].append(lambda: proj_qk_chain("k", 1, 0, "v"))
    filler[1].append(lambda: proj_qk_chain("k", 1, 1, "s"))
    filler[2].append(lambda: proj_qk_chain("q", 0, 3, "v"))
    filler[2].append(lambda: proj_qk_chain("k", 1, 2, "s"))
    filler[2].append(lambda: proj_qk_chain("k", 1, 3, "v"))
    for sc in range(SC):
        filler[3].append(lambda sc=sc: proj_qk_chain("q", 1, sc,
                                                     "s" if sc % 2 else "v"))
    for st in range(8):
        filler[4].append(lambda st=st: proj_v_chain(st, 1))
    for i in range(5, 8):
        for r in range(2):
            filler[i].append(lambda: junk_mms(2))

    prev = None  # (pair, qc, ax_tiles, y_ps)
    for i, (pair, qc) in enumerate(blocks):
        ax_tiles = [ax_pool.tile([128, 1024], BF16, tag="ax", name="ax")
                    for _kt in range(STL)]
        y_ps = []
        for hh in range(2):
            yp = psy_pool.tile([128, 4, HD + 1], F32, tag=f"y{hh}",
                               name=f"y{hh}")
            y_ps.append(yp)
        pv_prev = pv_mms(prev[0], prev[1], prev[2], prev[3]) if prev else []
        assert len(pv_prev) in (0, 128)
        gen = qk_exp_block(pair, qc, ax_tiles)
        fq = filler[i]
        for step in range(STL // 2):
            if pv_prev:
                # 32-PV batches every other step: fewer 64<->128-row
                # tiling-mode transitions on the PE
                if step % 2 == 0:
                    for mm in pv_prev[16 * step: 16 * (step + 2)]:
                        mm()
                if step == 3:
                    epilogue_h(prev[0], prev[1], prev[3], 0)
                if step == 7:
                    epilogue_h(prev[0], prev[1], prev[3], 1)
                if fq and step % 2 == 0:
                    fq.popleft()()
            elif fq and (step % 2 == 1 or len(fq) > (STL // 2 - step)):
                fq.popleft()()
            next(gen)
        while fq:
            fq.popleft()()
        prev = (pair, qc, ax_tiles, y_ps)
        if i == 0:
            for st in range(6, STL):
                proj_v_chain(st, 0)
        if i == 4:
            for st in range(8, STL):
                proj_v_chain(st, 1)
    # drain the last block
    pv_last = pv_mms(prev[0], prev[1], prev[2], prev[3])
    for n, mm in enumerate(pv_last):
        mm()
        if n == 63:
            epilogue_h(prev[0], prev[1], prev[3], 0)
    epilogue_h(prev[0], prev[1], prev[3], 1)


def _build():
    if "nc" in _compiled:
        return _compiled["nc"]
    nc = bacc.Bacc("TRN2", target_bir_lowering=False, debug=False,
                   num_devices=N_CORES)
    from contextlib import ExitStack
    with tile.TileContext(nc) as tc, ExitStack() as ctx:
        _emit(tc, ctx)
    nc.compile()
    _compiled["nc"] = nc
    return nc


def kernel(x, Wq, bq, Wk, bk, Wv, bv, _profile=False):
    x = np.asarray(x, dtype=np.float32)
    Wq = np.asarray(Wq, dtype=np.float32)
    Wk = np.asarray(Wk, dtype=np.float32)
    Wv = np.asarray(Wv, dtype=np.float32)
    bq = np.asarray(bq, dtype=np.float32)
    bk = np.asarray(bk, dtype=np.float32)
    bv = np.asarray(bv, dtype=np.float32)

    nc = _build()

    bf = ml_dtypes.bfloat16
    xT = [np.ascontiguousarray(x[b].T).astype(bf) for b in range(B)]
    in_maps = []
    for c in range(N_CORES):
        b, g = divmod(c, HPC)
        sl = slice(g * C, (g + 1) * C)
        in_maps.append({
            "xT": xT[b],
            "wq": np.ascontiguousarray(Wq[:, sl]).astype(bf),
            "wk": np.ascontiguousarray(Wk[:, sl]).astype(bf),
            "wv": np.ascontiguousarray(Wv[:, sl]).astype(bf),
            "bq": np.ascontiguousarray(bq[sl]),
            "bk": np.ascontiguousarray(bk[sl]),
            "bv": np.ascontiguousarray(bv[sl]),
        })

    from concourse.bass_utils import run_bass_kernel_spmd

    if _profile:
        _install_ntff_hook()
    res = run_bass_kernel_spmd(nc, in_maps, list(range(N_CORES)),
                               trace=_profile)
    out = np.empty((B, S, D), dtype=np.float32)
    for c in range(N_CORES):
        b, g = divmod(c, HPC)
        out[b, :, g * C: (g + 1) * C] = res.results[c]["y"]
    if _profile:
        kernel.last_exec_time_ns = res.exec_time_ns
    return out


# revision 32
# speedup vs baseline: 1.0054x; 1.0054x over previous
"""Trainium2 Bass kernel for nn_CausalSelfAttention_22127671509246.

Full (unsharded) inputs in, full output out. Internally shards across 8
NeuronCores: core c handles batch b = c // 4 and head group g = c % 4
(heads 4g..4g+3, i.e. a 256-wide slice of the QKV output channels).

v3 design (per core, 4 heads = 2 head pairs):
  - Q^T/K^T projections into flat [128,512] chunks (channel-major), V
    projection row-major in fp16 with a ones column per head (PV matmul
    then also produces the softmax denominator).
  - attention blocks (pair, qc): QK in 64-row matmuls (2 cols/cycle via
    row replication); logits exp'd on TWO engines:
      * 9/16 k-tiles: ScalarE Exp activation -> bf16
      * 7/16 k-tiles: DVE Schraudolph (i16 = round(l*A+B), bitcast fp16
        ~= exp(l); softmax normalization cancels most of the ~3% elem
        error -> ~0.9% output err)
  - PV matmuls of block i-1 interleave per-kp-step into block i's QK
    stream; per-head epilogues at steps 3/7 release PSUM early.
  - ALL projection work (V halves, ct1 Q/K, late Q ct0) is spread as
    per-step PE filler so the PE never idles and the HAM clock-gate
    stays at 2.4 GHz; junk matmuls keep it warm when filler runs dry.
"""

import sys
import types
from collections import deque

sys.path.insert(0, "/opt/trn_rl_repo")

import numpy as np
import ml_dtypes

import concourse.bass as bass
import concourse.bacc as bacc
import concourse.mybir as mybir
import concourse.tile as tile
from concourse.bass import ts

B, S, D = 2, 2048, 1024
H, HD = 16, 64
N_CORES = 8
C = 256           # output channels per core (4 heads)
CT = C // 128     # channel tiles per core
KD = D // 128     # contraction chunks for the projections
SC = S // 512     # 512-wide column chunks of S
STL = S // 128    # 128-row tiles of S
HPC = 4           # heads per core
SCALE = 1.0 / np.sqrt(HD)

LOG2E = float(np.log2(np.e))
SCH_A = SCALE * LOG2E * 1024.0          # fold softmax scale into schraudolph
SCH_B = (15.0 - 0.043) * 1024.0         # fp16 bias + optimal shift

F32 = mybir.dt.float32
BF16 = mybir.dt.bfloat16
FP16 = mybir.dt.float16
I16 = mybir.dt.int16

# k-tiles handled by the DVE (schraudolph); the rest by ScalarE exp
DVE_KT = frozenset({1, 3, 5, 7, 9, 11, 13, 15})

_compiled = {}


def _install_ntff_hook():
    if "antenv.axon_hooks" in sys.modules:
        return
    try:
        import trn_agent_boot.trn_boot as tb

        mod = types.ModuleType("antenv.axon_hooks")
        hook = tb._ntff_profile_via_ctypes("/opt/axon/libaxon_pjrt.so")
        mod.get_axon_ntff_profile_hook = lambda: hook
        mod.set_axon_ntff_profile_hook = lambda h: None
        sys.modules["antenv.axon_hooks"] = mod
    except Exception:
        pass


def _emit(tc, ctx):
    nc = tc.nc
    xT = nc.dram_tensor("xT", [D, S], BF16, kind="ExternalInput").ap()
    wq = nc.dram_tensor("wq", [D, C], BF16, kind="ExternalInput").ap()
    wk = nc.dram_tensor("wk", [D, C], BF16, kind="ExternalInput").ap()
    wv = nc.dram_tensor("wv", [D, C], BF16, kind="ExternalInput").ap()
    bq = nc.dram_tensor("bq", [C], F32, kind="ExternalInput").ap()
    bk = nc.dram_tensor("bk", [C], F32, kind="ExternalInput").ap()
    bv = nc.dram_tensor("bv", [C], F32, kind="ExternalInput").ap()
    y = nc.dram_tensor("y", [S, C], F32, kind="ExternalOutput").ap()

    singles = ctx.enter_context(tc.tile_pool(name="singles", bufs=1))
    ax_pool = ctx.enter_context(tc.tile_pool(name="ax", bufs=34))
    yout_pool = ctx.enter_context(tc.tile_pool(name="yout", bufs=4))
    recip_pool = ctx.enter_context(tc.tile_pool(name="recip", bufs=4))
    ps_pool = ctx.enter_context(tc.tile_pool(name="ps", bufs=3, space="PSUM"))
    psy_pool = ctx.enter_context(tc.tile_pool(name="psy", bufs=1, space="PSUM"))

    # ---- input DMAs: fine chunks, alternating the two HWDGE queues ----
    xT_r = xT.rearrange("(o p) s -> p o s", p=128)
    xT_sb = singles.tile([128, KD, S], BF16)
    w_sbs = {
        "q": singles.tile([128, KD, C], BF16, tag="wq", name="wq_sb"),
        "k": singles.tile([128, KD, C], BF16, tag="wk", name="wk_sb"),
        "v": singles.tile([128, KD, C], BF16, tag="wv", name="wv_sb"),
    }
    bq_sb = singles.tile([128, CT], F32, tag="bq")
    bk_sb = singles.tile([128, CT], F32, tag="bk")

    # FEW input DMA transfers: each dma_start costs ~2us of issue +
    # completion-semaphore latency per queue, so the critical path must
    # cross as few semaphores as possible
    nc.sync.dma_start(w_sbs["k"][:], wk.rearrange("(o p) c -> p o c", p=128))
    nc.scalar.dma_start(w_sbs["q"][:], wq.rearrange("(o p) c -> p o c", p=128))
    nc.sync.dma_start(xT_sb[:, 0:2, :], xT_r[:, 0:2, :])
    nc.scalar.dma_start(xT_sb[:, 2:4, :], xT_r[:, 2:4, :])
    nc.sync.dma_start(xT_sb[:, 4:6, :], xT_r[:, 4:6, :])
    nc.scalar.dma_start(xT_sb[:, 6:8, :], xT_r[:, 6:8, :])
    nc.sync.dma_start(w_sbs["v"][:], wv.rearrange("(o p) c -> p o c", p=128))
    # strided many-descriptor bias loads go through the gpsimd software
    # DGE so they don't block the HWDGE queues' in-order completions
    nc.gpsimd.dma_start(out=bk_sb[:], in_=bk.rearrange("(o p) -> p o", p=128))
    nc.gpsimd.dma_start(out=bq_sb[:], in_=bq.rearrange("(o p) -> p o", p=128))
    # bv broadcast across partitions (DMA with partition step 0)
    bv_bc = singles.tile([128, C], F32, tag="bvbc")
    bv_bcast_ap = bass.AP(tensor=bv.tensor, offset=bv.offset,
                          ap=[[0, 128]] + list(bv.ap))
    nc.gpsimd.dma_start(out=bv_bc[:], in_=bv_bcast_ap)

    # ---- PE warm-up junk matmuls (HAM clock-gate to 8/8) ----
    junk = singles.tile([128, 512], BF16, tag="junk")
    nc.vector.memset(junk[:], 0.0)

    def junk_mms(n):
        ps = ps_pool.tile([128, 512], F32, tag="qk", name="warm")
        for r in range(n):
            nc.tensor.matmul(ps[:], lhsT=junk[:, 0:128], rhs=junk[:],
                             start=True, stop=True, skip_group_check=True)

    junk_mms(24)

    # V with a ones column appended per head: [128, s_tile, head, 65] fp16
    v_sb = singles.tile([128, STL, HPC, HD + 1], FP16, tag="vones")
    nc.vector.memset(v_sb[:, :, :, HD], 1.0)

    # flat projection chunk tiles
    qch = [[singles.tile([128, 512], BF16, tag=f"qc{ct}{sc}",
                         name=f"qc{ct}{sc}")
            for sc in range(SC)] for ct in range(CT)]
    kch = [[singles.tile([128, 512], BF16, tag=f"kc{ct}{sc}",
                         name=f"kc{ct}{sc}")
            for sc in range(SC)] for ct in range(CT)]

    def proj_qk_chain(which, ct, sc, eng="v"):
        w_sb = w_sbs[which]
        dst = (qch if which == "q" else kch)[ct][sc]
        bias = bq_sb if which == "q" else bk_sb
        ps = ps_pool.tile([128, 512], F32, tag="qk", name="ps_proj")
        for kd in range(KD):
            nc.tensor.matmul(
                ps[:],
                lhsT=w_sb[:, kd, ts(ct, 128)],
                rhs=xT_sb[:, kd, ts(sc, 512)],
                start=(kd == 0),
                stop=(kd == KD - 1),
            )
        if eng == "v":
            nc.vector.tensor_scalar_add(dst[:], ps[:], bias[:, ct:ct + 1])
        else:
            nc.scalar.activation(dst[:], ps[:],
                                 mybir.ActivationFunctionType.Identity,
                                 bias=bias[:, ct:ct + 1], scale=1.0)

    def proj_v_chain(st, pair):
        """V projection for one 128-row s-tile, one head PAIR (N=128)."""
        ps = ps_pool.tile([128, 128], F32, tag="qk", name="ps_projv")
        for kd in range(KD):
            nc.tensor.matmul(
                ps[:],
                lhsT=xT_sb[:, kd, ts(st, 128)],
                rhs=w_sbs["v"][:, kd, ts(pair, 128)],
                start=(kd == 0),
                stop=(kd == KD - 1),
            )
        nc.vector.tensor_tensor(
            v_sb[:, st, 2 * pair:2 * pair + 2, 0:HD],
            ps[:].rearrange("p (h d) -> p h d", h=2),
            bv_bc[:, ts(pair, 128)].rearrange("p (h d) -> p h d", h=2),
            mybir.AluOpType.add,
        )

    # ---- attention blocks ----
    blocks = [(pair, qc) for pair in range(HPC // 2) for qc in range(SC)]

    def qk_exp_block(pair, qc, ax_tiles):
        """Per kp step: QK(u0), exp(u0), QK(u1), exp(u1)."""
        ct = pair
        for kp in range(STL // 2):
            for u in range(2):
                kt = 2 * kp + u
                psu = ps_pool.tile([128, 1024], F32, tag="qk", name="ps_att")
                for hh in range(2):
                    p0 = hh * 64
                    nc.tensor.matmul(
                        psu[:, ts(hh, 512)],
                        lhsT=kch[ct][kt // 4][p0:p0 + 64, ts(kt % 4, 128)],
                        rhs=qch[ct][qc][p0:p0 + 64, :],
                        start=True,
                        stop=True,
                    )
                if kt in DVE_KT:
                    nc.vector.tensor_scalar(
                        ax_tiles[kt].bitcast(I16)[:], psu[:],
                        SCH_A, SCH_B,
                        mybir.AluOpType.mult, mybir.AluOpType.add,
                    )
                else:
                    nc.scalar.activation(
                        ax_tiles[kt][:], psu[:],
                        mybir.ActivationFunctionType.Exp, scale=SCALE,
                    )
            yield

    def pv_mms(pair, qc, ax_tiles, y_ps):
        mms = []
        for hh in range(2):
            h = 2 * pair + hh
            for j in range(4):
                for kt in range(STL):
                    def mm(hh=hh, h=h, j=j, kt=kt):
                        axt = ax_tiles[kt]
                        sl = slice(hh * 512 + j * 128,
                                   hh * 512 + (j + 1) * 128)
                        lhsT = (axt.bitcast(FP16)[:, sl] if kt in DVE_KT
                                else axt[:, sl])
                        nc.tensor.matmul(
                            y_ps[hh][:, j, :],
                            lhsT=lhsT,
                            rhs=v_sb[:, kt, h, :],
                            start=(kt == 0),
                            stop=(kt == STL - 1),
                        )
                    mms.append(mm)
        return mms

    def epilogue_h(pair, qc, y_ps, hh):
        h = 2 * pair + hh
        yo = yout_pool.tile([128, 4, HD], F32, tag="yo", name="yo")
        rc = recip_pool.tile([128, 4], F32, tag="rc", name="rc")
        nc.vector.reciprocal(rc[:], y_ps[hh][:, :, HD])
        nc.vector.tensor_tensor(
            yo[:],
            y_ps[hh][:, :, 0:HD],
            rc[:, :, None].to_broadcast((128, 4, HD)),
            mybir.AluOpType.mult,
        )
        nc.sync.dma_start(
            y[ts(qc, 512), ts(h, HD)].rearrange("(j p) d -> p j d", p=128),
            yo[:],
        )

    # lead-in: just K ct0 sc0 + Q ct0 sc0 (all block 0 needs to start)
    proj_qk_chain("k", 0, 0)
    proj_qk_chain("q", 0, 0)

    # per-block PE filler chains (each ~0.4-1.7us of matmul work).
    # In blocks with PV batches they run right after a 128-row PV batch so
    # they don't add extra 64<->128-row mode transitions.
    filler = {i: deque() for i in range(len(blocks))}
    filler[0].append(lambda: proj_qk_chain("k", 0, 1, "s"))
    filler[0].append(lambda: proj_qk_chain("q", 0, 1, "v"))
    filler[0].append(lambda: proj_qk_chain("k", 0, 2, "s"))
    filler[0].append(lambda: proj_qk_chain("k", 0, 3, "v"))
    for st in range(6):
        filler[0].append(lambda st=st: proj_v_chain(st, 0))
    filler[1].append(lambda: proj_qk_chain("q", 0, 2, "s"))
    filler[1].append(lambda: proj_qk_chain("k", 1, 0, "v"))
    filler[1].append(lambda: proj_qk_chain("k", 1, 1, "s"))
    filler[2].append(lambda: proj_qk_chain("q", 0, 3, "v"))
    filler[2].append(lambda: proj_qk_chain("k", 1, 2, "s"))
    filler[2].append(lambda: proj_qk_chain("k", 1, 3, "v"))
    for sc in range(SC):
        filler[3].append(lambda sc=sc: proj_qk_chain("q", 1, sc,
                                                     "s" if sc % 2 else "v"))
    for st in range(8):
        filler[4].append(lambda st=st: proj_v_chain(st, 1))
    for i in range(5, 8):
        for r in range(2):
            filler[i].append(lambda: junk_mms(2))

    prev = None  # (pair, qc, ax_tiles, y_ps)
    for i, (pair, qc) in enumerate(blocks):
        ax_tiles = [ax_pool.tile([128, 1024], BF16, tag="ax", name="ax")
                    for _kt in range(STL)]
        y_ps = []
        for hh in range(2):
            yp = psy_pool.tile([128, 4, HD + 1], F32, tag=f"y{hh}",
                               name=f"y{hh}")
            y_ps.append(yp)
        pv_prev = pv_mms(prev[0], prev[1], prev[2], prev[3]) if prev else []
        assert len(pv_prev) in (0, 128)
        gen = qk_exp_block(pair, qc, ax_tiles)
        fq = filler[i]
        for step in range(STL // 2):
            if pv_prev:
                # 32-PV batches every other step: fewer 64<->128-row
                # tiling-mode transitions on the PE
                if step % 2 == 0:
                    for mm in pv_prev[16 * step: 16 * (step + 2)]:
                        mm()
                if step == 3:
                    epilogue_h(prev[0], prev[1], prev[3], 0)
                if step == 7:
                    epilogue_h(prev[0], prev[1], prev[3], 1)
                if fq and step % 2 == 0:
                    fq.popleft()()
            elif fq and (step % 2 == 1 or len(fq) > (STL // 2 - step)):
                fq.popleft()()
            next(gen)
        while fq:
            fq.popleft()()
        prev = (pair, qc, ax_tiles, y_ps)
        if i == 0:
            for st in range(6, STL):
                proj_v_chain(st, 0)
        if i == 4:
            for st in range(8, STL):
                proj_v_chain(st, 1)
    # drain the last block
    pv_last = pv_mms(prev[0], prev[1], prev[2], prev[3])
    for n, mm in enumerate(pv_last):
        mm()
        if n == 63:
            epilogue_h(prev[0], prev[1], prev[3], 0)
    epilogue_h(prev[0], prev[1], prev[3], 1)


def _build():
    if "nc" in _compiled:
        return _compiled["nc"]
    nc = bacc.Bacc("TRN2", target_bir_lowering=False, debug=False,
                   num_devices=N_CORES)
    from contextlib import ExitStack
    with tile.TileContext(nc) as tc, ExitStack() as ctx:
        _emit(tc, ctx)
    nc.compile()
    _compiled["nc"] = nc
    return nc


def kernel(x, Wq, bq, Wk, bk, Wv, bv, _profile=False):
    x = np.asarray(x, dtype=np.float32)
    Wq = np.asarray(Wq, dtype=np.float32)
    Wk = np.asarray(Wk, dtype=np.float32)
    Wv = np.asarray(Wv, dtype=np.float32)
    bq = np.asarray(bq, dtype=np.float32)
    bk = np.asarray(bk, dtype=np.float32)
    bv = np.asarray(bv, dtype=np.float32)

    nc = _build()

    bf = ml_dtypes.bfloat16
    xT = [np.ascontiguousarray(x[b].T).astype(bf) for b in range(B)]
    in_maps = []
    for c in range(N_CORES):
        b, g = divmod(c, HPC)
        sl = slice(g * C, (g + 1) * C)
        in_maps.append({
            "xT": xT[b],
            "wq": np.ascontiguousarray(Wq[:, sl]).astype(bf),
            "wk": np.ascontiguousarray(Wk[:, sl]).astype(bf),
            "wv": np.ascontiguousarray(Wv[:, sl]).astype(bf),
            "bq": np.ascontiguousarray(bq[sl]),
            "bk": np.ascontiguousarray(bk[sl]),
            "bv": np.ascontiguousarray(bv[sl]),
        })

    from concourse.bass_utils import run_bass_kernel_spmd

    if _profile:
        _install_ntff_hook()
    res = run_bass_kernel_spmd(nc, in_maps, list(range(N_CORES)),
                               trace=_profile)
    out = np.empty((B, S, D), dtype=np.float32)
    for c in range(N_CORES):
        b, g = divmod(c, HPC)
        out[b, :, g * C: (g + 1) * C] = res.results[c]["y"]
    if _profile:
        kernel.last_exec_time_ns = res.exec_time_ns
    return out
